# revision 63
# baseline (speedup 1.0000x reference)
"""Trainium2 Bass kernel for nn_Decoder (LSTM decoder + attention + lm_head).

Sharding: data-parallel over batch (64 -> 8 cores x 8). Each core runs the
full pipeline for its batch shard locally; no collectives.

Per-core pipeline (one NEFF), feature-major recurrence, software-pipelined:
  A) XGT[m] = W_ih[m-chunk] @ X.T for all steps (bf16, stationary weight
     chunks; gates land feature-major in 16 chunks)
  B) 63 sequential LSTM cell steps, feature-major (see baseline docstring for
     the tanh-as-sigmoid folding).  The per-step dependency-latency windows
     are filled with ready work:
       - lm_head weight-bank prefetch DMAs stream on the SP HWDGE queue into
         a deep SBUF ring (backpressured by ring-slot reuse)
       - attention (C) + out-projection (C2) for the previous 16-step token
         chunk are emitted between steps, so their PE/ACT/DVE work runs in
         the recurrence's stall windows
  C/C2) attention + out-proj per 128-token chunk (4 chunks)
  D) vocab projection (bf16): logits = OUT @ W_lm.T streamed over 32000
     vocab in 512-wide banks from the prefetch ring; fp16 eviction
     alternates ACT/DVE; output DMA on the ACT HWDGE queue.  b_lm is added
     host-side during output assembly (saves an 8.2MB broadcast-bias DMA).
"""
import sys

sys.path.insert(0, "/opt/trn_rl_repo")

import numpy as np
import ml_dtypes

from concourse import bacc, bass, mybir
from concourse.tile import TileContext
from concourse.bass_utils import run_bass_kernel_spmd

f32 = mybir.dt.float32
fp16 = mybir.dt.float16
bf16 = mybir.dt.bfloat16
Act = mybir.ActivationFunctionType
Alu = mybir.AluOpType

NCORES = 8
T = 63            # decode steps (tgt_len - 1)
BL = 8            # batch per core
TOK = T * BL      # 504 tokens per core
TOKP = 512        # padded
SRC = 128
HID = 512
ENC = 512
INP = 512
V = 32000
GATES = 4 * HID   # 2048
NBANK = (V + 511) // 512  # 63 vocab banks (last = 256 wide)
PF = 15           # lm_head weight prefetch ring depth (banks)
# token chunks (start step, #steps): 16/16/16 for the D-tile-aligned bulk,
# then 8/7 so the un-overlapped tail only carries a 56-token attention pass
CHUNKS = [(0, 16), (16, 16), (32, 16), (48, 8), (56, 7)]
NCH = len(CHUNKS)
STEP2CHUNK = {}
for _ci, (_t0, _n) in enumerate(CHUNKS):
    for _t in range(_t0, _t0 + _n):
        STEP2CHUNK[_t] = _ci

# torch gate order i,f,g,o -> pipeline order f,i,g,o
PERM = np.concatenate([np.arange(512, 1024), np.arange(0, 512),
                       np.arange(1024, 1536), np.arange(1536, 2048)])

_BF = ml_dtypes.bfloat16


def _chunk_steps(c):
    return CHUNKS[c]


def _build(niter: int = 1) -> "bacc.Bacc":
    nc = bacc.Bacc("TRN2", target_bir_lowering=False)

    xt_d = nc.dram_tensor("xt", [INP, TOKP], bf16, kind="ExternalInput")
    wih_d = nc.dram_tensor("wih", [INP, GATES], bf16, kind="ExternalInput")
    whh_d = nc.dram_tensor("whh", [HID, GATES], bf16, kind="ExternalInput")
    h0t_d = nc.dram_tensor("h0t", [128, 32], bf16, kind="ExternalInput")
    c0_d = nc.dram_tensor("c0", [128, 32], f32, kind="ExternalInput")
    enc_d = nc.dram_tensor("encf", [BL * SRC, ENC], f32, kind="ExternalInput")
    enct_d = nc.dram_tensor("enctf", [BL * ENC, SRC], f32, kind="ExternalInput")
    mbt_d = nc.dram_tensor("mbtT", [1, BL * SRC], f32, kind="ExternalInput")
    win_d = nc.dram_tensor("wint", [HID, ENC], mybir.dt.float32r, kind="ExternalInput")
    wout_d = nc.dram_tensor("woutt", [ENC + HID, HID], bf16, kind="ExternalInput")
    wlm_d = nc.dram_tensor("wlmt", [HID, V], bf16, kind="ExternalInput")
    id128_d = nc.dram_tensor("id128b", [128, 128], bf16, kind="ExternalInput")
    out_d = nc.dram_tensor("logits", [TOKP, V], fp16, kind="ExternalOutput")

    with TileContext(nc) as tc:
        for i in range(niter):
            if i:
                tc.strict_bb_all_engine_barrier()
            _emit_iter(nc, tc, xt_d, wih_d, whh_d, h0t_d, c0_d, enc_d, enct_d,
                       mbt_d, win_d, wout_d, wlm_d, id128_d, out_d)
    nc.compile()
    return nc


def _emit_iter(nc, tc, xt_d, wih_d, whh_d, h0t_d, c0_d, enc_d, enct_d, mbt_d,
               win_d, wout_d, wlm_d, id128_d, out_d):
    MM = nc.tensor.matmul

    with (
        tc.tile_pool(name="const", bufs=1) as cp,
        tc.tile_pool(name="store", bufs=1) as stp,
        tc.tile_pool(name="pd_w", bufs=PF) as pd_w,
        tc.tile_pool(name="pd_st", bufs=3) as pd_st,
    ):
        # ---- resident tiles ----
        whh = [cp.tile([128, GATES], bf16, tag=f"whh{k}", name=f"whh{k}") for k in range(4)]
        h0t = cp.tile([128, 32], bf16, tag="h0t", name="h0t")
        nc.sync.dma_start(out=h0t[:], in_=h0t_d[:])
        c0 = cp.tile([128, 32], f32, tag="c0", name="c0")
        nc.sync.dma_start(out=c0[:], in_=c0_d[:])
        id128 = cp.tile([128, 128], bf16, tag="id128", name="id128")
        nc.sync.dma_start(out=id128[:], in_=id128_d[:])
        # attention-phase constants: DMAs emitted after phase A so they
        # stream during the (DMA-idle) LSTM recurrence
        enc_sb = [cp.tile([SRC, ENC], f32, tag=f"enc{b}", name=f"enc{b}") for b in range(BL)]
        enctb = [cp.tile([128, 4 * SRC], f32, tag=f"enct{b}", name=f"enct{b}") for b in range(BL)]
        enct_sb = [[enctb[b][:, k * SRC:(k + 1) * SRC] for k in range(4)] for b in range(BL)]
        mbtT = cp.tile([1, BL * SRC], f32, tag="mbtT", name="mbtT")
        wint = [cp.tile([128, ENC], mybir.dt.float32r, tag=f"wint{k}", name=f"wint{k}") for k in range(4)]
        woutt = [cp.tile([128, HID], bf16, tag=f"woutt{k}", name=f"woutt{k}") for k in range(8)]

        def emit_attn_const_dmas():
            for b in range(BL):
                nc.sync.dma_start(out=enc_sb[b][:], in_=enc_d[b * SRC:(b + 1) * SRC, :])
            for b in range(BL):
                nc.sync.dma_start(
                    out=enctb[b][:].rearrange("p (k s) -> p k s", k=4),
                    in_=enct_d[b * ENC:(b + 1) * ENC, :].rearrange("(k p) s -> p k s", k=4))
            nc.sync.dma_start(out=mbtT[:], in_=mbt_d[:])
            for k in range(4):
                nc.sync.dma_start(out=wint[k][:], in_=win_d[k * 128:(k + 1) * 128, :])
            for k in range(8):
                nc.sync.dma_start(out=woutt[k][:], in_=wout_d[k * 128:(k + 1) * 128, :])

        ones_s = cp.tile([128, 1], f32, tag="ones_s", name="ones_s")
        nc.any.memset(ones_s[:], 1.0)
        ones_1 = cp.tile([1, 128], f32, tag="ones_1", name="ones_1")
        nc.any.memset(ones_1[:], 1.0)

        # ---- accumulating stores ----
        xgtB = stp.tile([128, TOKP * 16], bf16, tag="xgtB", name="xgtB")
        htbB = stp.tile([128, 4 * TOKP], bf16, tag="htbB", name="htbB")
        htb = [htbB[:, k * TOKP:(k + 1) * TOKP] for k in range(4)]
        # chunk-local b-major f32r h store, written directly by the
        # recurrence: chunk c cols c*128 + b*Tc + j  (j = local step)
        htfbmB = stp.tile([128, 4 * TOKP], mybir.dt.float32r, tag="htfbmB", name="htfbmB")
        qtfB = stp.tile([128, 4 * 128], f32, tag="qtfB", name="qtfB")  # per-chunk Q
        ctxt = [stp.tile([128, TOKP], bf16, tag=f"ctxt{k}", name=f"ctxt{k}") for k in range(4)]
        outt = [stp.tile([128, TOKP], bf16, tag=f"outt{m}", name=f"outt{m}") for m in range(4)]
        for m in range(4):
            # zero the 8 pad-token columns so phase D can use full 128-row tiles
            nc.any.memset(outt[m][:, TOK:TOKP], 0.0)

        # lm_head prefetch ring (filled by backpressured SP-queue DMAs)
        wl_ring = []

        # PSUM ring for the early vocab units (allocated before the phase-A
        # pools so pool releases stay LIFO)
        pe_ps = tc.alloc_tile_pool(name="pe_ps", bufs=2, space="PSUM")

        # ================= Phase A: XGT = (W_ih @ X.T) =================
        # chunked: only chunk 0's XGT is computed up front; later chunks'
        # groups are interleaved into B's stall windows (1 m-group each)
        pa_sb = tc.alloc_tile_pool(name="pa_sb", bufs=1)
        pa_ps = tc.alloc_tile_pool(name="pa_ps", bufs=1, space="PSUM")
        xts = [pa_sb.tile([128, TOKP], bf16, tag=f"xt{k}", name=f"xt{k}") for k in range(4)]
        for k in range(4):
            nc.sync.dma_start(out=xts[k][:], in_=xt_d[k * 128:(k + 1) * 128, :])
        wih = [pa_sb.tile([128, GATES], bf16, tag=f"wih{k}", name=f"wih{k}") for k in range(4)]
        for k in range(4):
            nc.sync.dma_start(out=wih[k][:], in_=wih_d[k * 128:(k + 1) * 128, :])
        for k in range(4):
            nc.sync.dma_start(out=whh[k][:], in_=whh_d[k * 128:(k + 1) * 128, :])

        def emit_a(c, m):
            t0, n = _chunk_steps(c)
            w = 8 * n
            ps = pa_ps.tile([128, 128], f32, tag="pa", name="pa")
            for k in range(4):
                MM(ps[:, 0:w], wih[k][:, m * 128:(m + 1) * 128],
                   xts[k][:, 8 * t0:8 * t0 + w],
                   start=(k == 0), stop=(k == 3))
            xgv = (xgtB[:].rearrange("p (t c) -> p t c", c=128)
                   [:, t0:t0 + n, m * 8:(m + 1) * 8])
            psv = ps[:, 0:w].rearrange("p (t b) -> p t b", b=8)
            # GPSIMD cannot read PSUM; split the evictions ACT/DVE
            if m % 2 == 0:
                nc.scalar.copy(xgv, psv)
            else:
                nc.vector.tensor_copy(xgv, psv)

        for m in range(16):
            emit_a(0, m)

        emit_attn_const_dmas()

        # lm_head weight-bank prefetch: all 63 DMAs emitted now on the SP
        # queue; ring-slot WAR dependencies pace them behind phase D's
        # consumption, so the first PF banks stream during the recurrence.
        for nb in range(NBANK):
            n0 = nb * 512
            nw = min(512, V - n0)
            wl4 = pd_w.tile([128, 4 * 512], bf16, tag="wl4", name="wl4")
            nc.sync.dma_start(
                out=wl4[:].rearrange("p (k n) -> p k n", k=4)[:, :, 0:nw],
                in_=wlm_d[:, n0:n0 + nw].rearrange("(k p) n -> p k n", k=4))
            wl_ring.append(wl4)

        # ============ attention (C) + out-proj (C2) chunk emitters ============
        def emit_q(c):
            t0, tc_ = _chunk_steps(c)
            w = 8 * tc_
            with tc.tile_pool(name="pq_ps", bufs=1, space="PSUM") as pq_ps:
                for m in range(4):
                    ps = pq_ps.tile([128, w], f32, tag="q", name="q")
                    for k in range(4):
                        MM(ps[:], wint[k][:, m * 128:(m + 1) * 128],
                           htfbmB[:, k * TOKP + 8 * t0:k * TOKP + 8 * t0 + w],
                           start=(k == 0), stop=(k == 3))
                    nc.vector.tensor_copy(qtfB[:, m * 128:m * 128 + w], ps[:])

        def make_attn(c):
            # returns a list of emit-closures for chunk c's attention
            t0, tc_ = _chunk_steps(c)
            w = 8 * tc_
            pools = {}

            def open_pools():
                pools["sb"] = tc.alloc_tile_pool(name="pc_sb", bufs=1)
                pools["s"] = tc.alloc_tile_pool(name="pc_s", bufs=1, space="PSUM")
                pools["pss"] = pools["s"].tile([SRC, w], f32, tag="scores", name="scores")
                pools["e_all"] = pools["sb"].tile([SRC, w], f32, tag="e_all", name="e_all")

            def scores(b):
                # 4 contraction MMs + a K=1 rank-one MM folding the softmax
                # mask bias into the scores PSUM (so exp needs no per-b bias)
                pss = pools["pss"]
                for k in range(4):
                    MM(pss[:, b * tc_:(b + 1) * tc_], enct_sb[b][k],
                       qtfB[:].rearrange("p (m j) -> p m j", m=4)[:, k, b * tc_:(b + 1) * tc_],
                       start=(k == 0), stop=False)
                MM(pss[:, b * tc_:(b + 1) * tc_],
                   mbtT[:, b * SRC:(b + 1) * SRC],
                   ones_1[:, 0:tc_], start=False, stop=True)

            def exps():
                # one unbiased exp over the whole chunk: single ACT op,
                # one act-table switch away from the sigmoid set and one back
                pss, e_all = pools["pss"], pools["e_all"]
                nc.scalar.activation(e_all[:, 0:w], pss[:, 0:w], Act.Exp)
                pools["s"].release()  # scores PSUM bank free for denom/ctx
                pools["c"] = tc.alloc_tile_pool(name="pc_c", bufs=1, space="PSUM")

            def denom():
                e_all = pools["e_all"]
                psd = pools["c"].tile([1, w], f32, tag="denom", name="denom")
                MM(psd[:], ones_s[:], e_all[:], start=True, stop=True)
                rec = pools["sb"].tile([1, w], f32, tag="rec", name="rec")
                nc.vector.reciprocal(rec[:], psd[:])
                psb = pools["c"].tile([128, w], f32, tag="recb_ps", name="recb_ps")
                MM(psb[:], ones_1[:], rec[:], start=True, stop=True)
                recb = pools["sb"].tile([128, w], f32, tag="recb", name="recb")
                nc.vector.tensor_copy(recb[:], psb[:])
                pools["recb"] = recb

            def ctx(k):
                e_all, recb = pools["e_all"], pools["recb"]
                psc = pools["c"].tile([128, w], f32, tag="ctx", name="ctx")
                for b in range(BL):
                    MM(psc[:, b * tc_:(b + 1) * tc_],
                       enc_sb[b][:, k * 128:(k + 1) * 128],
                       e_all[:, b * tc_:(b + 1) * tc_], start=True, stop=True)
                # normalize + scatter b-major -> token-major in one strided mul
                ctxv = (ctxt[k][:, 8 * t0:8 * t0 + w]
                        .rearrange("p (j b) -> p b j", b=BL))
                pscv = psc[:].rearrange("p (b j) -> p b j", b=BL)
                recv = recb[:].rearrange("p (b j) -> p b j", b=BL)
                nc.vector.tensor_mul(ctxv, pscv, recv)

            def close_pools():
                pools["c"].release()
                pools["sb"].release()

            ops = [lambda: (open_pools(), scores(0), scores(1))[0] and None,
                   lambda: (scores(2), scores(3)),
                   lambda: (scores(4), scores(5)),
                   lambda: (scores(6), scores(7)),
                   exps,
                   denom,
                   lambda: (ctx(0), ctx(1)),
                   lambda: (ctx(2), ctx(3), close_pools())]
            return ops

        def emit_c2(c, m):
            t0, tc_ = _chunk_steps(c)
            w = 8 * tc_
            with tc.tile_pool(name="po_ps", bufs=1, space="PSUM") as po_ps:
                ps = po_ps.tile([128, w], f32, tag="o", name="o")
                for k in range(8):
                    rhs = ctxt[k] if k < 4 else htb[k - 4]
                    MM(ps[:], woutt[k][:, m * 128:(m + 1) * 128],
                       rhs[:, 8 * t0:8 * t0 + w],
                       start=(k == 0), stop=(k == 7))
                # tanh(x) = 2*sig(2x)-1: outt holds sig(2x); the affine fix
                # (2x weight scale + column-sum bias shift) is folded into
                # wlmt/host bias, keeping the ACT engine in the sigmoid set
                nc.scalar.activation(outt[m][:, 8 * t0:8 * t0 + w], ps[:],
                                     Act.Sigmoid, scale=2.0)

        # per-step deferred-work schedule: step t -> list of closures
        sched = {}

        def add(t, fn):
            sched.setdefault(t, []).append(fn)

        for c in range(NCH - 1):
            # chunk c's C/C2 interleaved into chunk c+1's steps
            base, nnext = _chunk_steps(c + 1)
            ops = [lambda c=c: emit_q(c)]
            ops += make_attn(c)
            ops += [lambda c=c, m=m: emit_c2(c, m) for m in range(4)]
            for i, op in enumerate(ops):
                add(base + (i * nnext) // len(ops), op)

        # ---- early vocab-projection units ----
        # The prefetched weight banks get (bank, mt) units executed inside
        # the recurrence's idle windows: the PE MMs fill its stall time, and
        # the output DMA rides the idle SWDGE queue — removing both PE and
        # DMA time from phase D.
        def emit_unit(nb, mt, engine="pool"):
            n0 = nb * 512
            nw = min(512, V - n0)
            ps = pe_ps.tile([128, 512], f32, tag="eu", name="eu")
            for k in range(4):
                MM(ps[:, 0:nw], outt[k][:, mt * 128:(mt + 1) * 128],
                   wl_ring[nb][:, k * 512:k * 512 + nw],
                   start=(k == 0), stop=(k == 3))
            st = pd_st.tile([128, 512], fp16, tag="ste", name="ste")
            # GPSIMD cannot read PSUM: evict on ACT/DVE (alternating), but
            # the out-DMA rides the idle SWDGE (Pool) queue during B
            if (nb + mt) % 2 == 0:
                nc.scalar.copy(st[:, 0:nw], ps[:, 0:nw])
            else:
                nc.vector.tensor_copy(st[:, 0:nw], ps[:, 0:nw])
            if engine == "pool":
                nc.gpsimd.dma_start(out=out_d[mt * 128:(mt + 1) * 128, n0:n0 + nw],
                                    in_=st[:, 0:nw])
            else:
                nc.scalar.dma_start(out=out_d[mt * 128:(mt + 1) * 128, n0:n0 + nw],
                                    in_=st[:, 0:nw])

        early_set = set()

        def fill(slots, units):
            it = iter(units)
            for s, k in slots:
                for _ in range(k):
                    u = next(it, None)
                    if u is None:
                        return
                    early_set.add(u)
                    add(s, lambda u=u: emit_unit(*u))

        # XGT for chunks 1..4 interleaved so each chunk's groups land before
        # the recurrence reaches it
        for m in range(16):
            add(m, lambda m=m: emit_a(1, m))
            add(16 + m, lambda m=m: emit_a(2, m))
            add(32 + m // 2, lambda m=m: emit_a(3, m))
            add(48 + m // 2, lambda m=m: emit_a(4, m))

        # placement respects when C2 of the needed token chunk lands
        fill([(30, 2), (31, 2)] + [(s, 1) for s in range(32, 46)],
             [(nb, 0) for nb in range(PF)])
        fill([(46, 2), (47, 2)] + [(s, 1) for s in range(48, 56)] + [(56, 2), (57, 2)],
             [(nb, 1) for nb in range(PF)])
        fill([(s, 2) for s in range(58, 63)],
             [(nb, 2) for nb in range(10)])

        # ================= Phase B: LSTM recurrence (feature-major) ======
        with (
            tc.tile_pool(name="pb_tmp", bufs=3) as pb_tmp,
            tc.tile_pool(name="pb_c", bufs=2) as pb_c,
            tc.tile_pool(name="pb_g", bufs=2, space="PSUM") as pb_g,
        ):
            c_prev = c0
            for t in range(T):
                c_ = STEP2CHUNK[t]
                t0c, tc_ = _chunk_steps(c_)
                # gatesT in one PSUM tile: chunk m at cols m*8..m*8+8
                # layout: f = cols 0:32, i = 32:64, g = 64:96, o = 96:128
                G = pb_g.tile([128, 128], f32, tag="G", name="G")
                MM(G[:], id128[:], xgtB[:, t * 128:(t + 1) * 128],
                   start=True, stop=False)
                for m in range(16):
                    for k in range(4):
                        rhs = (h0t[:, k * 8:(k + 1) * 8] if t == 0 else
                               htbB[:, k * TOKP + (t - 1) * 8:
                                    k * TOKP + t * 8])
                        MM(G[:, m * 8:(m + 1) * 8],
                           whh[k][:, m * 128:(m + 1) * 128], rhs,
                           start=False, stop=(k == 3))
                # pointwise, all feature-major. ACT ops all Sigmoid
                # (tanh(x) = 2*sigmoid(2x)-1; the *2 of h is pre-folded into
                # W_hh/W_in/W_out host-side, so h is stored as h/2).
                sall = pb_tmp.tile([128, 128], f32, tag="sall", name="sall")
                # one sigmoid over all four gate regions: the ~185ns ACT
                # access latency is per-instruction, so one wide op beats
                # three narrow ones (ACT is the recurrence's busiest engine)
                nc.scalar.activation(sall[:], G[:], Act.Sigmoid)
                c1 = pb_tmp.tile([128, 32], f32, tag="c1", name="c1")
                nc.vector.tensor_mul(c1[:], sall[:, 0:32], c_prev[:])
                # up = (sig(2g) - 0.5) * sig(i)  == sig(i)*tanh(g)/2
                up = pb_tmp.tile([128, 32], f32, tag="up", name="up")
                nc.vector.scalar_tensor_tensor(
                    up[:], sall[:, 64:96], 0.5, sall[:, 32:64],
                    op0=Alu.subtract, op1=Alu.mult)
                c_new = pb_c.tile([128, 32], f32, tag="c", name="c")
                nc.vector.scalar_tensor_tensor(
                    c_new[:], up[:], 2.0, c1[:],
                    op0=Alu.mult, op1=Alu.add)
                tc2 = pb_tmp.tile([128, 32], f32, tag="tc2", name="tc2")
                nc.scalar.activation(tc2[:], c_new[:], Act.Sigmoid, scale=2.0)
                # h/2 = (sig(2c) - 0.5) * sig(o), stored feature-major
                tv = tc2[:].rearrange("p (k c) -> p k c", k=4)
                sv = sall[:, 96:128].rearrange("p (k c) -> p k c", k=4)
                hbv = htbB[:].rearrange("p (k c) -> p k c", k=4)[:, :, t * 8:(t + 1) * 8]
                hfv = (htfbmB[:].rearrange("p (k r) -> p k r", k=4)
                       [:, :, 8 * t0c:8 * t0c + 8 * tc_]
                       .rearrange("p k (b j) -> p k b j", b=BL)[:, :, :, t - t0c])
                # hfv first: the next step's matmuls wait on the LAST h write,
                # so the token-major hbv (their actual input) must be it
                nc.vector.scalar_tensor_tensor(
                    hfv, tv, 0.5, sv, op0=Alu.subtract, op1=Alu.mult)
                nc.vector.scalar_tensor_tensor(
                    hbv, tv, 0.5, sv, op0=Alu.subtract, op1=Alu.mult)
                c_prev = c_new
                # deferred work for earlier chunks fills this step's window
                for fn in sched.get(t, []):
                    fn()

        pa_ps.release()
        pa_sb.release()

        # ---- tail: last chunk's attention + out-proj ----
        emit_q(NCH - 1)
        for op in make_attn(NCH - 1):
            op()
        for m in range(4):
            emit_c2(NCH - 1, m)

        # ================= Phase D: vocab projection =================
        # PSUM pool opened only now: B/C pools are closed, so 4 banks fit
        pd_ps = tc.alloc_tile_pool(name="pd_ps", bufs=4, space="PSUM")
        for nb in range(NBANK):
            n0 = nb * 512
            nw = min(512, V - n0)
            wl4 = wl_ring[nb]
            rem = [mt for mt in range(4) if (nb, mt) not in early_set]
            if len(rem) < 4:
                # prefetched bank partially done during B: per-unit path
                for mt in rem:
                    emit_unit(nb, mt, engine="dve")
                continue
            st4 = pd_st.tile([128, 4 * 512], fp16, tag="st4", name="st4")
            for mt in range(4):
                m0 = mt * 128
                ps = pd_ps.tile([128, 512], f32, tag="v", name="v")
                for k in range(4):
                    MM(ps[:, 0:nw], outt[k][:, m0:m0 + 128],
                       wl4[:, k * 512:k * 512 + nw],
                       start=(k == 0), stop=(k == 3))
                # evictions alternate ACT/DVE to split the copy bandwidth
                if mt % 2 == 0:
                    nc.scalar.copy(st4[:, mt * 512:mt * 512 + nw], ps[:, 0:nw])
                else:
                    nc.vector.tensor_copy(st4[:, mt * 512:mt * 512 + nw], ps[:, 0:nw])
            # one DMA writes all 512 (padded) token rows of this bank, on the
            # ACT HWDGE queue so it doesn't head-of-line block the SP-queue
            # weight prefetch stream
            nc.scalar.dma_start(
                out=out_d[:, n0:n0 + nw].rearrange("(m p) n -> p m n", m=4),
                in_=st4[:].rearrange("p (m n) -> p m n", m=4)[:, :, 0:nw])
        pd_ps.release()
        pe_ps.release()


def _prep_in_maps(inputs: dict) -> list[dict]:
    targets = np.asarray(inputs["targets"])
    mask = np.asarray(inputs["attention_mask"])
    enc = np.asarray(inputs["encodings"], dtype=np.float32)
    h = np.asarray(inputs["h"], dtype=np.float32)
    c = np.asarray(inputs["c"], dtype=np.float32)
    emb = np.asarray(inputs["emb"], dtype=np.float32)
    W_ih = np.asarray(inputs["W_ih"], dtype=np.float32)
    W_hh = np.asarray(inputs["W_hh"], dtype=np.float32)
    W_in = np.asarray(inputs["W_in"], dtype=np.float32)
    W_out = np.asarray(inputs["W_out"], dtype=np.float32)
    W_lm = np.asarray(inputs["W_lm"], dtype=np.float32)

    x_seq = emb[targets[:-1]]                      # (63, 64, 512)
    # gate-g rows doubled so the single on-chip sigmoid yields sig(2g)
    # (tanh(g) = 2*sig(2g) - 1); h is stored as h/2 on-chip, so W_hh is
    # doubled once more for every gate.
    wih_g2 = W_ih[PERM].copy()
    wih_g2[1024:1536] *= 2.0
    wih_p = wih_g2.T.astype(_BF).copy()            # (512, 2048)
    whh_g2 = (2.0 * W_hh)[PERM].copy()
    whh_g2[1024:1536] *= 2.0
    whh_p = whh_g2.T.astype(_BF).copy()
    wint = (2.0 * W_in).T.astype(np.float32).copy()  # (512, 512)
    W_out2 = W_out.copy()
    W_out2[:, ENC:] *= 2.0
    woutt = W_out2.T.astype(_BF).copy()            # (1024, 512)
    # outt holds sig(2x) = (tanh(x)+1)/2, so scale W_lm by 2 (the -1 shift
    # becomes a host-side column-sum bias correction in _assemble)
    wlmt = (2.0 * W_lm).T.astype(_BF).copy()       # (512, 32000)
    id128b = np.eye(128, dtype=_BF)

    def fmajor(a, dtype):
        # (BL, 512) batch-major -> feature-major [128, 4*8] chunk layout
        at = np.ascontiguousarray(a.T).reshape(4, 128, BL)
        return np.concatenate([at[k] for k in range(4)], axis=1).astype(dtype)

    in_maps = []
    for cidx in range(NCORES):
        sl = slice(cidx * BL, (cidx + 1) * BL)
        xt = np.zeros((INP, TOKP), np.float32)
        xt[:, :TOK] = x_seq[:, sl, :].reshape(TOK, INP).T
        h0t = fmajor(h[sl] * 0.5, _BF)                            # h0/2
        c0t = fmajor(c[sl], np.float32)
        encc = enc[:, sl, :]                                      # (128, 8, 512)
        encf = np.ascontiguousarray(encc.transpose(1, 0, 2)).reshape(BL * SRC, ENC)
        enctf = np.ascontiguousarray(encc.transpose(1, 2, 0)).reshape(BL * ENC, SRC)
        mbtT = np.ascontiguousarray(
            np.where(mask[:, sl], np.float32(-1e30), np.float32(0.0)).T
        ).astype(np.float32).reshape(1, BL * SRC)
        in_maps.append({
            "xt": xt.astype(_BF),
            "wih": wih_p, "whh": whh_p,
            "h0t": h0t,
            "c0": c0t,
            "encf": encf.astype(np.float32),
            "enctf": enctf.astype(np.float32),
            "mbtT": mbtT,
            "wint": wint, "woutt": woutt, "wlmt": wlmt,
            "id128b": id128b,
        })
    return in_maps


def _assemble(results, b_lm, w_colsum) -> np.ndarray:
    out = np.empty((T, 64, V), np.float32)
    for cidx in range(NCORES):
        lg = results[cidx]["logits"][:TOK].astype(np.float32).reshape(T, BL, V)
        out[:, cidx * BL:(cidx + 1) * BL, :] = lg
    out += (b_lm - w_colsum).astype(np.float32)
    return out


_CACHE: dict = {}


def kernel(**inputs) -> np.ndarray:
    if "nc" not in _CACHE:
        _CACHE["nc"] = _build(niter=1)
    in_maps = _prep_in_maps(inputs)
    res = run_bass_kernel_spmd(_CACHE["nc"], in_maps, core_ids=list(range(NCORES)))
    w_colsum = np.asarray(inputs["W_lm"], dtype=np.float32).sum(axis=1)
    return _assemble(res.results, np.asarray(inputs["b_lm"], dtype=np.float32),
                     w_colsum)
